# revision 2
# baseline (speedup 1.0000x reference)
"""Trainium2 Bass kernel for MemoryEfficientFlashAttention (B=2,S=2048,HID=2048,H=16,HKV=8,D=128,CHUNK=512).

Sharding: 8 cores = 2 batches x 4 head-groups (4 q heads / 2 kv heads per core).
Each core computes q/k/v projections (+RoPE), the chunked flash-attention
recurrence, and a row-sharded partial of the output projection (transposed).
Host sums the 4 partials per batch and adds bo.

Math: the reference's scan step is algebraically
    o_j = (o_{j-1} * e^{m_{j-1}} + Y_j) / (e^{m_{j-1}} + S_j)
with Y_j = exp(sc_j) @ V_j, S_j = rowsum exp(sc_j), m_j = running max.
Unrolled with the trailing o/d divide:
    o = sum_j (RawE_j @ V) * c_j,   RawE_j = exp(sc_j)  (raw, no max subtract;
    scores are O(6) so exp cannot overflow),
    c_j[q] = 1 / (M_n * prod_{l>=j} d_l * d_n^flag),  d_l = (M_{l-1} + S_l)/M_l,
    M_l = running max of exp scores (M_{-1} = 0), flag = processed the
    globally-last kv chunk (reproduces the reference's final o/d divide).

Pass 1 computes scores [q,k] once per chunk, exp's them into a RESIDENT bf16
P tile (plus Act-accumulated raw row sums and DVE row maxes); the (M, S) chain
runs entirely in the exp domain on DVE (mul/max/reciprocal - no Ln/Exp chain
ops).  Pass 2 never recomputes scores: each 128x128 P block is multiplied by
diag(c_t) on the tensor engine (a plain matmul with a diagonal rhs built via
one tensor_scalar per block column), which transposes AND scales in one pass;
the fp32 PSUM result is copied to bf16 SBUF (Act/DVE) and fed to the PV
matmuls accumulating u = sum_j c_j*(E_j^T) @ V directly in PSUM.

Perf structure: bf16 operands for all large matmuls, causal narrowing of
diagonal chunks, single shared 128x128 triangular mask tile, weights resident
in SBUF, v-projection run early on the still-loaded x chunk (no reloads), and
per-(qi,h) pass1->pass2 pipelining so only ~2 P tiles are ever live.
"""

import os
import sys
from contextlib import ExitStack

import numpy as np
import ml_dtypes

sys.path.insert(0, "/opt/trn_rl_repo")
os.environ.setdefault("MYCRO_LOCAL_CACHE", "1")

import concourse.bass as bass  # noqa: E402
import concourse.tile as tile  # noqa: E402
from concourse import bacc, mybir  # noqa: E402
from concourse.bass_utils import run_bass_kernel_spmd  # noqa: E402

# Steer insert_act_table_loads to a table set that holds Exp (and Copy),
# so the kernel loads one activation table total.
import collections  # noqa: E402
import concourse.hw_specs as _hw_specs  # noqa: E402

_gat_orig = _hw_specs.get_activation_tables


def _gat_combined(arch):
    tabs = _gat_orig(arch)
    both = {mybir.ActivationFunctionType.Exp, mybir.ActivationFunctionType.Ln}
    out = collections.OrderedDict()
    for name, s in tabs.items():
        if name == "natural_log_exp_and_others" or not (s & both):
            out[name] = s
        else:
            out[name] = s - both
    return out


bacc.get_activation_tables = _gat_combined

B, S, HID = 2, 2048, 2048
H, HKV, D = 16, 8, 128
CHUNK = 512
THETA = 1000000.0
NEG = -1e9
NCORES = 8
HL = H // (NCORES // B)      # 4 local q heads
KVL = HKV // (NCORES // B)   # 2 local kv heads
NQ = S // CHUNK              # 4 chunks
NT = HID // 128              # 16 hid tiles
SCALE = 1.0 / np.sqrt(np.float32(D))

F32 = mybir.dt.float32
F32R = mybir.dt.float32r
BF16 = mybir.dt.bfloat16
Alu = mybir.AluOpType
Act = mybir.ActivationFunctionType
BFNP = ml_dtypes.bfloat16

_CACHE = {}


def _rope_tables():
    inv_freq = 1.0 / (THETA ** (np.arange(0, D, 2, dtype=np.float32) / D))
    pos = np.arange(S, dtype=np.float32)
    freqs = pos[:, None].astype(np.float32) * inv_freq[None, :]
    emb = np.concatenate([freqs, freqs], axis=-1)  # [S, D]
    cosT = np.cos(emb).astype(np.float32).T.copy()
    sinT = np.sin(emb).astype(np.float32).T.copy()
    return cosT, sinT  # [D, S]


def _classify_mask(attention_mask):
    """Per (qi, j) CHUNKxCHUNK block: 'zero' | 'neg' | 'tri' (canonical causal
    diagonal), merged across batches so the SPMD program is identical on all
    cores. Only pure-causal masks are supported by this kernel."""
    q = np.arange(CHUNK)
    tri_full = np.where(q[:, None] >= q[None, :], 0.0, NEG).astype(np.float32)
    kinds = {}
    for qi in range(NQ):
        for j in range(NQ):
            kind = None
            for b in range(B):
                blk = attention_mask[b, 0, qi * CHUNK:(qi + 1) * CHUNK,
                                     j * CHUNK:(j + 1) * CHUNK]
                if np.all(blk == 0.0):
                    k = "zero"
                elif np.all(blk <= -1e6):
                    k = "neg"
                elif np.array_equal(blk, tri_full):
                    k = "tri"
                else:
                    raise NotImplementedError("non-causal mask block")
                if kind is None:
                    kind = k
                elif kind != k:
                    raise NotImplementedError("mask differs across batches")
            kinds[(qi, j)] = kind
    plan = {}
    for qi in range(NQ):
        processed = []
        for j in range(NQ):
            k = kinds[(qi, j)]
            if k == "neg" and len(processed) > 0:
                continue  # identity step under the reference's fp32 exp underflow
            assert k != "neg" or len(processed) == 0
            if k == "neg":
                # leading fully-masked chunk: contributes T=0 rows; unsupported
                raise NotImplementedError("leading all-neg chunk")
            processed.append((j, k == "tri"))
        plan[qi] = processed
    return plan


def _mm(nc, out, lhsT, rhs, start, stop):
    nc.tensor.matmul(out, lhsT, rhs, start=start, stop=stop)


def _emit(tc, ap, plan):
    nc = tc.nc

    with ExitStack() as top:
        # ---------------- persistent tensors ----------------
        pers = top.enter_context(tc.tile_pool(name="pers", bufs=1))
        KT = pers.tile([128, KVL, S], BF16)            # rope'd k^T  [d, kv, s]
        V = pers.tile([128, S // 128, KVL * D], BF16)  # v natural [s_p, s_t, kv*d]
        xt_pool = top.enter_context(tc.tile_pool(name="xt", bufs=2))
        qt_pool = top.enter_context(tc.tile_pool(name="qtp", bufs=2))
        hsT_r = ap["hsT"].rearrange("(t p) s -> p t s", p=128)

        xts = {}

        def load_xt(sq):
            xt = xt_pool.tile([128, NT, CHUNK], BF16, tag="xt")
            ssl = slice(sq * CHUNK, (sq + 1) * CHUNK)
            for tq in range(4):
                nc.sync.dma_start(xt[:, tq * 4:(tq + 1) * 4, :],
                                  hsT_r[:, tq * 4:(tq + 1) * 4, ssl])
            xts[sq] = xt

        # startup DMAs ordered by first use: first-half weights + first x
        # chunk + rope tables first, everything else behind them
        wqk_sb = pers.tile([128, NT, (HL + KVL) * 128], BF16)
        wqk_r = ap["wqk"].rearrange("(t p) m -> p t m", p=128)
        ssl0 = slice(0, CHUNK)
        xt0 = xt_pool.tile([128, NT, CHUNK], BF16, tag="xt")
        xts[0] = xt0
        nc.sync.dma_start(wqk_sb[:, :2], wqk_r[:, :2])
        nc.sync.dma_start(xt0[:, :2, :], hsT_r[:, :2, ssl0])
        nc.sync.dma_start(wqk_sb[:, 2:4], wqk_r[:, 2:4])
        nc.sync.dma_start(xt0[:, 2:4, :], hsT_r[:, 2:4, ssl0])
        for tq in range(1, 4):
            nc.sync.dma_start(wqk_sb[:, tq * 4:(tq + 1) * 4],
                              wqk_r[:, tq * 4:(tq + 1) * 4])
            nc.sync.dma_start(xt0[:, tq * 4:(tq + 1) * 4, :],
                              hsT_r[:, tq * 4:(tq + 1) * 4, ssl0])
        cosT = pers.tile([128, S], BF16)
        sinT = pers.tile([128, S], BF16)
        nc.sync.dma_start(cosT[:, ssl0], ap["cosT"][:, ssl0])
        nc.sync.dma_start(sinT[:, ssl0], ap["sinT"][:, ssl0])
        R128 = pers.tile([128, 128], F32R)
        nc.sync.dma_start(R128, ap["rmat"])
        bqk = pers.tile([128, HL + KVL], F32)
        nc.sync.dma_start(bqk, ap["bqk"])
        wv_sb = pers.tile([128, NT, KVL * D], BF16)
        nc.sync.dma_start(wv_sb[:, :4], ap["wv"].rearrange("(t p) m -> p t m", p=128)[:, :4])
        for cq in range(1, NQ):
            cs = slice(cq * CHUNK, (cq + 1) * CHUNK)
            nc.sync.dma_start(cosT[:, cs], ap["cosT"][:, cs])
            nc.sync.dma_start(sinT[:, cs], ap["sinT"][:, cs])
        nc.sync.dma_start(wv_sb[:, 4:], ap["wv"].rearrange("(t p) m -> p t m", p=128)[:, 4:])
        bv = pers.tile([1, KVL * D], F32R)
        nc.sync.dma_start(bv, ap["bv"])
        ones1 = pers.tile([1, 128], F32R)
        nc.sync.dma_start(ones1, ap["ones1"])
        I128b = pers.tile([128, 128], BF16)
        nc.sync.dma_start(I128b, ap["imatb"])
        triN = pers.tile([128, 128], BF16)
        nc.sync.dma_start(triN, ap["triN"])
        wo_sb = pers.tile([128, HL, HID], BF16)
        wo_r = ap["wo"].rearrange("(t p) m -> p t m", p=128)
        for mo in range(4):
            nc.sync.dma_start(wo_sb[:, :, mo * 512:(mo + 1) * 512],
                              wo_r[:, :, mo * 512:(mo + 1) * 512])

        # ---------------- pools (single scope; PSUM budget = 8 banks) ------
        raw_pool = top.enter_context(tc.tile_pool(name="raw", bufs=2))
        t_pool = top.enter_context(tc.tile_pool(name="ropetmp", bufs=2))
        ps_proj = top.enter_context(tc.tile_pool(name="psproj", bufs=3, space="PSUM"))
        ps_att = top.enter_context(tc.tile_pool(name="psatt", bufs=2, space="PSUM"))
        ps_t = top.enter_context(tc.tile_pool(name="pst", bufs=2, space="PSUM"))
        u_ps = top.enter_context(tc.tile_pool(name="ups", bufs=1, space="PSUM"))

        p_pool = top.enter_context(tc.tile_pool(name="pstore", bufs=2))
        ch_pool = top.enter_context(tc.tile_pool(name="chain", bufs=2))
        d_pool = top.enter_context(tc.tile_pool(name="diags", bufs=2))
        p2_pool = top.enter_context(tc.tile_pool(name="pprime", bufs=5))
        o2_pool = top.enter_context(tc.tile_pool(name="uout", bufs=2))
        o_pool = top.enter_context(tc.tile_pool(name="osb", bufs=4))

        QTs = {}

        def proj_qk(sq):
            ssl = slice(sq * CHUNK, (sq + 1) * CHUNK)
            xt = xts[sq]
            if sq + 1 < NQ:
                load_xt(sq + 1)
            QT = qt_pool.tile([128, HL, CHUNK], BF16, tag="qt", name=f"qt{sq}")
            QTs[sq] = QT

            # q^T and k^T projections, rope'd; the R-matmul + elementwise
            # rope tail run one m behind the qk accumulation so the PE never
            # waits on the Pool-engine bias add
            def rope_tail(m, raw):
                pr = ps_proj.tile([128, CHUNK], F32, tag="pp")
                _mm(nc, pr, R128, raw, start=True, stop=True)
                t1 = t_pool.tile([128, CHUNK], F32, tag="t1")
                nc.gpsimd.tensor_mul(t1, raw.bitcast(F32), cosT[:, ssl])
                t2 = t_pool.tile([128, CHUNK], F32, tag="t2")
                nc.vector.tensor_mul(t2, pr, sinT[:, ssl])
                dest = QT[:, m, :] if m < HL else KT[:, m - HL, ssl]
                nc.vector.tensor_add(dest, t1, t2)

            pend_rope = []
            for m in range(HL + KVL):
                ps = ps_proj.tile([128, CHUNK], F32, tag="pp")
                for t in range(NT):
                    _mm(nc, ps, wqk_sb[:, t, m * 128:(m + 1) * 128], xt[:, t],
                        start=(t == 0), stop=(t == NT - 1))
                raw = raw_pool.tile([128, CHUNK], F32R)
                nc.vector.tensor_scalar_add(raw, ps, bqk[:, m:m + 1])
                pend_rope.append((m, raw))
                if len(pend_rope) > 1:
                    rope_tail(*pend_rope.pop(0))
            for item in pend_rope:
                rope_tail(*item)

        def proj_v(sq):
            # v projection (natural layout), bias via K=1 matmul; runs on the
            # still-loaded x chunk right after the qk projection
            xt = xts.pop(sq)
            for ss in range(CHUNK // 128):
                pv = ps_proj.tile([128, CHUNK], F32, tag="pp")
                for t in range(NT):
                    _mm(nc, pv[:, :KVL * D], xt[:, t, ss * 128:(ss + 1) * 128], wv_sb[:, t],
                        start=(t == 0), stop=False)
                _mm(nc, pv[:, :KVL * D], ones1, bv, start=False, stop=True)
                nc.vector.tensor_copy(V[:, sq * 4 + ss, :], pv[:, :KVL * D])

        # ---- pass1 for one (qi, h): scores once, exp into resident P,
        # raw sums via Act accumulate, row maxes via DVE; exp-domain chain ----
        def pass1_unit(qi, h):
            chunks = plan[qi]
            nj = len(chunks)
            QT = QTs[qi]
            P = p_pool.tile([128, 4, nj, CHUNK], BF16, tag="P",
                            name=f"P{qi}_{h}")
            mxe = ch_pool.tile([128, nj, 4], F32, tag="mxe", name=f"mxe{qi}_{h}")
            sraw = ch_pool.tile([128, nj, 4], F32, tag="sraw", name=f"sr{qi}_{h}")
            for t, (j, diag) in enumerate(chunks):
                k0 = j * CHUNK
                for sub in range(4):
                    q0 = sub * 128
                    w = (sub + 1) * 128 if diag else CHUNK
                    ps = ps_att.tile([128, CHUNK], F32, tag="ps")
                    _mm(nc, ps[:, :w], QT[:, h, q0:q0 + 128],
                        KT[:, h // 2, k0:k0 + w],
                        start=True, stop=not diag)
                    if diag:
                        _mm(nc, ps[:, w - 128:w], I128b, triN,
                            start=False, stop=True)
                    nc.scalar.activation(
                        P[:, sub, t, :w], ps[:, :w], Act.Exp,
                        accum_out=sraw[:, t, sub:sub + 1])
                    nc.vector.tensor_reduce(
                        mxe[:, t, sub:sub + 1], P[:, sub, t, :w],
                        axis=mybir.AxisListType.X, op=Alu.max)
            return {"qi": qi, "h": h, "nj": nj, "chunks": chunks,
                    "P": P, "mxe": mxe, "sraw": sraw}

        def chain_unit(st):
            # exp-domain chain on DVE:
            #   M_t = running max of mxe  (M_{-1} = 0)
            #   d_t = (M_{t-1} + S_t) / M_t
            #   c_t = 1 / (M_fin * prod_{l>=t} d_l * d_last^flag)
            qi, h, nj = st["qi"], st["h"], st["nj"]
            mxe, sraw = st["mxe"], st["sraw"]
            Mrun = ch_pool.tile([128, nj + 1, 4], F32, tag="Mrun",
                                name=f"Mr{qi}_{h}")
            nc.vector.memset(Mrun[:, 0, :], 0.0)
            for t in range(nj):
                nc.vector.tensor_tensor(Mrun[:, t + 1, :], Mrun[:, t, :],
                                        mxe[:, t, :], Alu.max)
            num = ch_pool.tile([128, nj, 4], F32, tag="num", name=f"nm{qi}_{h}")
            nc.vector.tensor_add(num, Mrun[:, :nj, :], sraw)
            rM = ch_pool.tile([128, nj, 4], F32, tag="rM", name=f"rM{qi}_{h}")
            nc.vector.reciprocal(rM, Mrun[:, 1:, :])
            dq = ch_pool.tile([128, nj, 4], F32, tag="dq", name=f"dq{qi}_{h}")
            nc.vector.tensor_mul(dq, num, rM)
            if any(j == NQ - 1 for (j, _) in st["chunks"]):
                nc.vector.tensor_mul(dq[:, nj - 1, :], dq[:, nj - 1, :],
                                     dq[:, nj - 1, :])
            # suffix products G_t = M_fin * prod_{l>=t} d_l
            G = ch_pool.tile([128, nj + 1, 4], F32, tag="G", name=f"G{qi}_{h}")
            nc.vector.tensor_copy(G[:, nj, :], Mrun[:, nj, :])
            for t in range(nj - 1, -1, -1):
                nc.vector.tensor_mul(G[:, t, :], dq[:, t, :], G[:, t + 1, :])
            cc = ch_pool.tile([128, nj, 4], F32, tag="cc", name=f"cc{qi}_{h}")
            nc.vector.reciprocal(cc, G[:, :nj, :])
            # diag(c) tiles for the transpose-scale matmuls
            Dg = d_pool.tile([128, nj, 4, 128], BF16, tag="Dg",
                             name=f"Dg{qi}_{h}")
            for t in range(nj):
                for sub in range(4):
                    nc.vector.tensor_scalar_mul(Dg[:, t, sub, :], I128b,
                                                cc[:, t, sub:sub + 1])
            st["Dg"] = Dg

        def pass2_unit(st, fill=()):
            # u[d, q] = sum_{t,kc} V_slab^T @ (P_block^T diag(c_t)) ; the
            # diag-matmul transposes AND scales P in one PE pass.
            fill = list(fill)
            qi, h, nj = st["qi"], st["h"], st["nj"]
            P, Dg = st["P"], st["Dg"]
            up = u_ps.tile([128, CHUNK], F32, tag="up", name=f"up{qi}_{h}")
            steps = [(t, j, diag, kc)
                     for t, (j, diag) in enumerate(st["chunks"])
                     for kc in range(4)]
            nstep = len(steps)

            LAG = 3
            pend = []

            def emit_pv(idx, item):
                j, kc, off, ptb = item
                _mm(nc, up[:, off:],
                    V[:, j * 4 + kc, (h // 2) * D:(h // 2 + 1) * D],
                    ptb[:, off:], start=(idx == 0), stop=(idx == nstep - 1))

            for i, (t, j, diag, kc) in enumerate(steps):
                off = kc * 128 if diag else 0
                pt = ps_t.tile([128, CHUNK], F32, tag="pt")
                for sub in range(kc if diag else 0, 4):
                    _mm(nc, pt[:, sub * 128:(sub + 1) * 128],
                        P[:, sub, t, kc * 128:(kc + 1) * 128],
                        Dg[:, t, sub, :], start=True, stop=True)
                ptb = p2_pool.tile([128, CHUNK], BF16)
                nc.scalar.activation(ptb[:, off:], pt[:, off:], Act.Copy)
                pend.append((i, (j, kc, off, ptb)))
                if len(pend) > LAG:
                    emit_pv(*pend.pop(0))
                if fill and i % 2 == 1:
                    fill.pop(0)()
            for item in pend:
                emit_pv(*item)
            ub = o2_pool.tile([128, CHUNK], BF16, tag=f"ub{h}",
                              name=f"ub{h}_{qi}")
            nc.vector.tensor_copy(ub, up)
            for f in fill:
                f()
            return ub

        def wo_unit(qi, ubs, mo):
            # one output-projection tile
            qsl = slice(qi * CHUNK, (qi + 1) * CHUNK)
            po = ps_proj.tile([128, CHUNK], F32, tag="pp")
            for t in range(HL):
                _mm(nc, po, wo_sb[:, t, mo * 128:(mo + 1) * 128], ubs[t],
                    start=(t == 0), stop=(t == HL - 1))
            ob = o_pool.tile([128, CHUNK], BF16)
            nc.vector.tensor_copy(ob, po)
            nc.sync.dma_start(ap["outT"][mo * 128:(mo + 1) * 128, qsl], ob)

        # ---------------- schedule ----------------
        # per qi: projections (PE-heavy) first, then per-h pass1 (Act/DVE
        # heavy) pipelined with pass2 (PE transposes + PV, Act copies);
        # wo(qi-1) units interleave as PE filler inside pass2 units.
        ub_store = {}
        wo_fill = []
        for qi in range(NQ):
            proj_qk(qi)
            proj_v(qi)
            ubs = []
            for h in range(HL):
                st = pass1_unit(qi, h)
                chain_unit(st)
                nfill = 2 if h < HL - 1 else len(wo_fill)
                fills = [wo_fill.pop(0) for _ in range(min(nfill, len(wo_fill)))]
                ubs.append(pass2_unit(st, fill=fills))
            ub_store[qi] = ubs
            wo_fill.extend(
                (lambda mo=mo, qi=qi, ubs=ubs: wo_unit(qi, ubs, mo))
                for mo in range(HID // 128))
        for f in wo_fill:
            f()


def _build_program(plan):
    nc = bacc.Bacc("TRN2", target_bir_lowering=False, debug=False,
                   enable_asserts=False, num_devices=NCORES)
    ap = {}
    ap["hsT"] = nc.dram_tensor("hsT", [HID, S], BF16, kind="ExternalInput").ap()
    ap["wqk"] = nc.dram_tensor("wqk", [HID, (HL + KVL) * D], BF16, kind="ExternalInput").ap()
    ap["wv"] = nc.dram_tensor("wv", [HID, KVL * D], BF16, kind="ExternalInput").ap()
    ap["wo"] = nc.dram_tensor("wo", [HL * D, HID], BF16, kind="ExternalInput").ap()
    ap["bqk"] = nc.dram_tensor("bqk", [D, HL + KVL], F32, kind="ExternalInput").ap()
    ap["bv"] = nc.dram_tensor("bv", [1, KVL * D], F32R, kind="ExternalInput").ap()
    ap["cosT"] = nc.dram_tensor("cosT", [D, S], BF16, kind="ExternalInput").ap()
    ap["sinT"] = nc.dram_tensor("sinT", [D, S], BF16, kind="ExternalInput").ap()
    ap["rmat"] = nc.dram_tensor("rmat", [D, D], F32R, kind="ExternalInput").ap()
    ap["imatb"] = nc.dram_tensor("imatb", [128, 128], BF16, kind="ExternalInput").ap()
    ap["triN"] = nc.dram_tensor("triN", [128, 128], BF16, kind="ExternalInput").ap()
    ap["ones1"] = nc.dram_tensor("ones1", [1, 128], F32R, kind="ExternalInput").ap()
    ap["outT"] = nc.dram_tensor("outT", [HID, S], BF16, kind="ExternalOutput").ap()

    with tile.TileContext(nc) as tc:
        _emit(tc, ap, plan)
    nc.compile()
    return nc


def _host_inputs(inputs):
    hs = np.asarray(inputs["hidden_states"], dtype=np.float32)
    Wq = np.asarray(inputs["Wq"], dtype=np.float32)
    bq = np.asarray(inputs["bq"], dtype=np.float32)
    Wk = np.asarray(inputs["Wk"], dtype=np.float32)
    bk = np.asarray(inputs["bk"], dtype=np.float32)
    Wv = np.asarray(inputs["Wv"], dtype=np.float32)
    bv_ = np.asarray(inputs["bv"], dtype=np.float32)
    Wo = np.asarray(inputs["Wo"], dtype=np.float32)

    cosT, sinT = _rope_tables()
    R = np.zeros((D, D), dtype=np.float32)
    R[64 + np.arange(64), np.arange(64)] = -1.0   # out[d'<64] = -q[d'+64]
    R[np.arange(64), 64 + np.arange(64)] = 1.0    # out[d'>=64] = q[d'-64]
    I = np.eye(128, dtype=np.float32)
    q = np.arange(128)
    triN = np.where(q[:, None] >= q[None, :], 0.0, NEG).astype(BFNP)

    Wq4 = (Wq * SCALE).reshape(HID, H, D)
    bq4 = (bq * SCALE).reshape(H, D)
    Wk4 = Wk.reshape(HID, HKV, D)
    bk4 = bk.reshape(HKV, D)
    Wv4 = Wv.reshape(HID, HKV, D)
    bv4 = bv_.reshape(HKV, D)
    Wo4 = Wo.reshape(H, D, HID)

    in_maps = []
    for c in range(NCORES):
        b, hg = divmod(c, NCORES // B)
        qh = slice(hg * HL, (hg + 1) * HL)
        kvh = slice(hg * KVL, (hg + 1) * KVL)
        wqk = np.concatenate([
            Wq4[:, qh].reshape(HID, HL * D),
            Wk4[:, kvh].reshape(HID, KVL * D)], axis=1)
        bqk = np.concatenate([bq4[qh], bk4[kvh]], axis=0).T  # [D, HL+KVL]
        in_maps.append({
            "hsT": hs[b].T.astype(BFNP),
            "wqk": wqk.astype(BFNP),
            "wv": Wv4[:, kvh].reshape(HID, KVL * D).astype(BFNP),
            "wo": Wo4[qh].reshape(HL * D, HID).astype(BFNP),
            "bqk": np.ascontiguousarray(bqk),
            "bv": bv4[kvh].reshape(1, KVL * D).copy(),
            "cosT": cosT.astype(BFNP),
            "sinT": sinT.astype(BFNP),
            "rmat": R,
            "imatb": I.astype(BFNP),
            "triN": triN,
            "ones1": np.ones((1, 128), dtype=np.float32),
        })
    return in_maps


def get_program(inputs):
    am = np.asarray(inputs["attention_mask"], dtype=np.float32)
    plan = _classify_mask(am)
    key = str(plan)
    if key not in _CACHE:
        _CACHE[key] = _build_program(plan)
    return _CACHE[key], plan, None


def run(inputs, **spmd_kwargs):
    nc, plan, _ = get_program(inputs)
    in_maps = _host_inputs(inputs)
    res = run_bass_kernel_spmd(nc, in_maps, core_ids=list(range(NCORES)),
                               **spmd_kwargs)
    bo = np.asarray(inputs["bo"], dtype=np.float32)
    out = np.empty((B, S, HID), dtype=np.float32)
    gpb = NCORES // B
    for b in range(B):
        acc = np.zeros((HID, S), dtype=np.float32)
        for c in range(b * gpb, (b + 1) * gpb):
            acc += np.asarray(res.results[c]["outT"]).astype(np.float32)
        out[b] = acc.T + bo
    return out, res


def kernel(**inputs) -> np.ndarray:
    out, _ = run(inputs)
    return out
